# revision 29
# baseline (speedup 1.0000x reference)
"""Trainium2 Bass kernel for nn_Attention_23476291240422 (sparse attention:
causal + 128-wide noncausal prefix block; b=4, n=2048, dim=2048, 16 heads,
d=128) distributed across 8 NeuronCores.

Sharding: head-parallel (2 heads/core) QKV projection + attention, AllToAll
(bf16) to flip head-sharded -> row-sharded, row-parallel out-proj (1024
rows/core). Inputs are staged to HBM as bf16 (the kernel computes in bf16
with fp32 PSUM accumulation). The AllToAll runs in two phases (seq-half
granularity) overlapped with attention/out-proj compute; stage 1 (projection)
is interleaved per batch with phase-A attention so ScalarE exp work hides
under TensorE projection matmuls.

kernel(x, w_qkv, w_out, b_out) -> [4, 2048, 2048] float32.
"""
import os
import sys
import types

import numpy as np
import ml_dtypes

import concourse.bass as bass
import concourse.mybir as mybir
import concourse.tile as tile
from concourse import bacc
from concourse import bass_utils

B, N, DIM = 4, 2048, 2048
HEADS, D, L = 16, 128, 128
W = 8
HPC = HEADS // W          # 2 heads per core
ROWS = B * N              # 8192
RPC = ROWS // W           # 1024 rows per core
SCALE = float(D) ** -0.5
P = 128
KT = DIM // P             # 16
S1CH = 512                # stage-1 seq chunk width
NBC = N // S1CH           # 4 stage-1 chunks per batch
CHW = 512                 # stage-2 i-chunk width / stage-4 col chunk
NJT = N // P              # 16 j-tiles
F32, BF16 = mybir.dt.float32, mybir.dt.bfloat16
PHASE_CHUNKS = ([1, 3], [0, 2])
M_BASE = (4, 0)  # stage-4 row-tile base per phase


def _install_ntff_hook():
    try:
        import antenv.axon_hooks  # noqa: F401
        return
    except ImportError:
        pass
    try:
        import antenv
        from trn_agent_boot.trn_boot import _ntff_profile_via_ctypes
        hook = [_ntff_profile_via_ctypes("/opt/axon/libaxon_pjrt.so")]
        mod = types.ModuleType("antenv.axon_hooks")
        mod.get_axon_ntff_profile_hook = lambda: hook[0]
        mod.set_axon_ntff_profile_hook = lambda h: hook.__setitem__(0, h)
        sys.modules["antenv.axon_hooks"] = mod
        antenv.axon_hooks = mod
    except Exception:
        pass


def build():
    nc = bacc.Bacc("TRN2", target_bir_lowering=False, debug=False, num_devices=W)

    xT = nc.dram_tensor("xT", [DIM, ROWS], BF16, kind="ExternalInput")
    wq = nc.dram_tensor("wq", [DIM, 6 * P], BF16, kind="ExternalInput")  # q0 q1 k0 k1 v0 v1
    wout = nc.dram_tensor("wout", [DIM, DIM], BF16, kind="ExternalInput")
    bout = nc.dram_tensor("bout", [1, DIM], F32, kind="ExternalInput")
    out = nc.dram_tensor("out", [RPC, DIM], F32, kind="ExternalOutput")

    tri_np = (np.arange(P)[:, None] <= np.arange(P)[None, :]).astype(ml_dtypes.bfloat16)
    tri = nc.inline_tensor(tri_np, name="tri")
    ident = nc.inline_tensor(np.eye(P, dtype=ml_dtypes.bfloat16), name="ident")

    def s1_batch(b, wq_bf, qk_b, v_dram_b, s1xf, s1ps, s1v_pool):
        xT_r = xT.rearrange("(kt p) n -> p kt n", p=P)
        for c in range(NBC):
            seq0 = b * N + c * S1CH
            xb = s1xf.tile([P, KT, S1CH], BF16, tag="xb")
            if b == 0 and c == 0:
                for kq in range(4):
                    nc.sync.dma_start(
                        xb[:, 4 * kq:4 * (kq + 1)],
                        xT_r[:, 4 * kq:4 * (kq + 1), seq0:seq0 + S1CH])
            else:
                nc.sync.dma_start(xb[:], xT_r[:, :, seq0:seq0 + S1CH])
            for m in range(4):
                ps = s1ps.tile([P, CHW], F32, tag="ps1")
                for kt in range(KT):
                    nc.tensor.matmul(
                        ps[:, :S1CH], wq_bf[:, kt, m * P:(m + 1) * P], xb[:, kt],
                        start=(kt == 0), stop=(kt == KT - 1))
                nc.vector.tensor_copy(
                    qk_b[:, m, c * S1CH:(c + 1) * S1CH], ps[:, :S1CH])
            for st2 in range(S1CH // P):
                st = c * (S1CH // P) + st2  # seq-tile within batch (0..15)
                psv = s1ps.tile([P, CHW], F32, tag="ps1")
                for kt in range(KT):
                    nc.tensor.matmul(
                        psv[:, :HPC * P], xb[:, kt, st2 * P:(st2 + 1) * P],
                        wq_bf[:, kt, 4 * P:6 * P],
                        start=(kt == 0), stop=(kt == KT - 1))
                vst = s1v_pool.tile([P, HPC * P], BF16, tag="vst")
                nc.vector.tensor_copy(vst[:], psv[:, :HPC * P])
                nc.sync.dma_start(v_dram_b[st * P:(st + 1) * P, :], vst[:])

    def s2_batch_phase(b, chunks, qk_b, v_dram_b, a2a_in_ph, pools,
                       tri_sb, ident_sb):
        s2va, s2pt, s2sm, s2ps, s2att, s2tp = pools
        v_r = v_dram_b.rearrange("(jt p) d -> p jt d", p=P)
        for hl in range(HPC):
            va = s2va.tile([P, NJT, P + 1], BF16, tag="va")
            nc.vector.memset(va[:, :, P:P + 1], 1.0)
            nc.sync.dma_start(va[:, :, :P], v_r[:, :, hl * P:(hl + 1) * P])
            qT = qk_b[:, hl]
            kTt = qk_b[:, 2 + hl]
            for c in chunks:
                pt = s2pt.tile([P, NJT, CHW], BF16, tag="pt")
                for J in range(4 * c + 4):
                    k_off = max(0, J - 4 * c)
                    nn_ = CHW - P * k_off
                    i0 = c * CHW + P * k_off
                    pss = s2ps.tile([P, CHW], F32, tag="pss")
                    nc.tensor.matmul(
                        pss[:, :nn_], kTt[:, J * P:(J + 1) * P],
                        qT[:, i0:(c + 1) * CHW], start=True, stop=True)
                    nc.scalar.activation(
                        pt[:, J, P * k_off:], pss[:, :nn_],
                        mybir.ActivationFunctionType.Exp, scale=SCALE)
                    if J >= 4 * c and not (c == 0 and J == 0):
                        nc.vector.tensor_mul(
                            pt[:, J, P * k_off:P * (k_off + 1)],
                            pt[:, J, P * k_off:P * (k_off + 1)], tri_sb[:])
                for pp in range(4):
                    it = 4 * c + pp
                    att = s2att.tile([P, P + 1], F32, tag="att")
                    for J in range(it + 1):
                        nc.tensor.matmul(
                            att[:], pt[:, J, P * pp:P * (pp + 1)], va[:, J],
                            start=(J == 0), stop=(J == it))
                    recip = s2sm.tile([P, 1], F32, tag="recip")
                    nc.vector.reciprocal(recip[:], att[:, P:P + 1])
                    attn = s2sm.tile([P, P], BF16, tag="attn")
                    nc.vector.tensor_scalar_mul(attn[:], att[:, :P], recip[:])
                    attT_ps = s2tp.tile([P, P], BF16, tag="attTps")
                    nc.tensor.transpose(attT_ps[:], attn[:], ident_sb[:])
                    attnT = s2sm.tile([P, P], BF16, tag="attnT")
                    nc.vector.tensor_copy(attnT[:], attT_ps[:])
                    dest = b * 2 + it // 8
                    cc = pp * P
                    last_w = nc.sync.dma_start(
                        a2a_in_ph[dest, hl * P:(hl + 1) * P, cc:cc + P],
                        attnT[:])
        return last_w

    with tile.TileContext(nc) as tc:
        with (
            tc.tile_pool(name="persist", bufs=1) as persist,
            tc.tile_pool(name="dram", bufs=1, space="DRAM") as dram,
        ):
            tri_sb = persist.tile([P, P], BF16)
            ident_sb = persist.tile([P, P], BF16)
            bout_sb = persist.tile([P, DIM], F32)

            v_drams = [dram.tile([N, HPC * P], BF16, name=f"v_dram{b}")
                       for b in range(B)]
            a2a_in = [dram.tile([W, HPC * P, RPC // 2], BF16, name=f"a2a_in{i}")
                      for i in range(2)]
            a2a_out = [dram.tile([W, HPC * P, RPC // 2], BF16, name=f"a2a_out{i}")
                       for i in range(2)]

            woutb_r = wout.rearrange("(kt p) c -> p kt c", p=P)

            with (
                tc.tile_pool(name="qk", bufs=1) as qkpool,
                tc.tile_pool(name="s2va", bufs=3) as s2va,
                tc.tile_pool(name="s2pt", bufs=2) as s2pt,
                tc.tile_pool(name="s2sm", bufs=8) as s2sm,
                tc.tile_pool(name="s2ps", bufs=2, space="PSUM") as s2ps,
                tc.tile_pool(name="s2att", bufs=2, space="PSUM") as s2att,
                tc.tile_pool(name="s2tp", bufs=2, space="PSUM") as s2tp,
            ):
                qk_bs = [qkpool.tile([P, 4, N], BF16, name=f"qkb{b}")
                         for b in range(B)]
                s2pools = (s2va, s2pt, s2sm, s2ps, s2att, s2tp)

                with (
                    tc.tile_pool(name="s1w", bufs=1) as s1w,
                    tc.tile_pool(name="s1v", bufs=3) as s1v_pool,
                    tc.tile_pool(name="s1xf", bufs=3) as s1xf,
                    tc.tile_pool(name="s1ps", bufs=2, space="PSUM") as s1ps,
                ):
                    wq_bf = s1w.tile([P, KT, 6 * P], BF16)
                    wq_r = wq.rearrange("(kt p) c -> p kt c", p=P)
                    nc.sync.dma_start(wq_bf[:, 0:4], wq_r[:, 0:4])
                    nc.sync.dma_start(tri_sb[:], tri.ap())
                    nc.sync.dma_start(ident_sb[:], ident.ap())
                    for kq in range(1, 4):
                        nc.sync.dma_start(
                            wq_bf[:, 4 * kq:4 * (kq + 1)],
                            wq_r[:, 4 * kq:4 * (kq + 1)])

                    # interleave: s1(b) then phase-A attention of batch b
                    for b in range(B):
                        s1_batch(b, wq_bf, qk_bs[b], v_drams[b],
                                 s1xf, s1ps, s1v_pool)
                        s2_batch_phase(b, PHASE_CHUNKS[0], qk_bs[b],
                                       v_drams[b], a2a_in[0], s2pools,
                                       tri_sb, ident_sb)

                nc.gpsimd.collective_compute(
                    "AllToAll", mybir.AluOpType.bypass,
                    replica_groups=[list(range(W))],
                    ins=[a2a_in[0][:].opt()], outs=[a2a_out[0][:].opt()],
                )

                with (
                    tc.tile_pool(name="s4l", bufs=2) as s4l,
                    tc.tile_pool(name="s4w", bufs=2) as s4w,
                    tc.tile_pool(name="s4o", bufs=4) as s4o,
                ):
                    # wb0/bias have no collective deps: load early.
                    wb0 = s4w.tile([P, KT, CHW], BF16, tag="wb")
                    nc.sync.dma_start(wb0[:], woutb_r[:, :, 0:CHW])
                    wb1 = s4w.tile([P, KT, CHW], BF16, tag="wb")
                    nc.sync.dma_start(wb1[:], woutb_r[:, :, CHW:2 * CHW])
                    nc.sync.dma_start(
                        bout_sb[:], bout.ap().to_broadcast((P, DIM)))

                    from concourse.tile import add_dep_helper
                    last_writes = []
                    lhs0 = None
                    for b in range(B):
                        lw = s2_batch_phase(b, PHASE_CHUNKS[1], qk_bs[b],
                                            v_drams[b], a2a_in[1], s2pools,
                                            tri_sb, ident_sb)
                        last_writes.append(lw)
                        if b == 1:
                            # lhs0 (waits collective A) pinned after batch-2's
                            # DMAs so it cannot head-of-line-block phase B.
                            lhs0 = s4l.tile([P, KT, RPC // 2], BF16, tag="lhs")
                            l0 = nc.sync.dma_start(
                                lhs0[:],
                                a2a_out[0].rearrange("w h r -> (w h) r")
                                .rearrange("(kt p) r -> p kt r", p=P))
                            add_dep_helper(
                                l0.ins, lw.ins, sync=False,
                                reason="order lhs0 after phase-B b1")
                    nc.gpsimd.collective_compute(
                        "AllToAll", mybir.AluOpType.bypass,
                        replica_groups=[list(range(W))],
                        ins=[a2a_in[1][:].opt()], outs=[a2a_out[1][:].opt()],
                    )


                    # ======== stage 4: out projection (2 phases) ========
                    with tc.tile_pool(name="s4ps", bufs=2,
                                      space="PSUM") as s4ps:
                        def s4_pass(phase, lhs, wbs):
                            last_out = None
                            for ncx in range(4):
                                if ncx < len(wbs):
                                    wb = wbs[ncx]
                                else:
                                    wb = s4w.tile([P, KT, CHW], BF16, tag="wb")
                                    nc.sync.dma_start(
                                        wb[:],
                                        woutb_r[:, :, ncx * CHW:(ncx + 1) * CHW])
                                for ml in range(4):
                                    m = M_BASE[phase] + ml
                                    ps4 = s4ps.tile([P, CHW], F32, tag="ps4")
                                    for kt in range(KT):
                                        nc.tensor.matmul(
                                            ps4[:],
                                            lhs[:, kt, ml * P:(ml + 1) * P],
                                            wb[:, kt],
                                            start=(kt == 0),
                                            stop=(kt == KT - 1))
                                    osb = s4o.tile([P, CHW], F32, tag="osb")
                                    nc.vector.tensor_tensor(
                                        osb[:], ps4[:],
                                        bout_sb[:, ncx * CHW:(ncx + 1) * CHW],
                                        mybir.AluOpType.add)
                                    last_out = nc.sync.dma_start(
                                        out[m * P:(m + 1) * P,
                                            ncx * CHW:(ncx + 1) * CHW],
                                        osb[:])
                            return last_out

                        last_out0 = s4_pass(0, lhs0, [wb0, wb1])
                        # preload phase-1's first weight chunk ahead of the
                        # collective-dependent lhs1 loads in the sync FIFO
                        wb10 = s4w.tile([P, KT, CHW], BF16, tag="wb")
                        nc.sync.dma_start(wb10[:], woutb_r[:, :, 0:CHW])
                        # lhs1 (waits collective B) pinned behind phase-0's
                        # final output write so it cannot block the sync FIFO
                        lhs1 = s4l.tile([P, KT, RPC // 2], BF16, tag="lhs")
                        a2a1_r = (a2a_out[1].rearrange("w h r -> (w h) r")
                                  .rearrange("(kt p) r -> p kt r", p=P))
                        for mlq in range(4):
                            l1 = nc.sync.dma_start(
                                lhs1[:, :, mlq * P:(mlq + 1) * P],
                                a2a1_r[:, :, mlq * P:(mlq + 1) * P])
                            add_dep_helper(
                                l1.ins, last_out0.ins, sync=False,
                                reason="order lhs1 after stage-4A outs")
                        s4_pass(1, lhs1, [wb10])

    nc.compile()
    return nc


_NC = None


def _get_nc():
    global _NC
    if _NC is None:
        _NC = build()
    return _NC


last_exec_time_ns = None
last_results = None


def kernel(x, w_qkv, w_out, b_out):
    global last_exec_time_ns, last_results
    _install_ntff_hook()
    nc = _get_nc()

    x = np.asarray(x, dtype=np.float32)
    w_qkv = np.asarray(w_qkv, dtype=np.float32)
    w_out = np.asarray(w_out, dtype=np.float32)
    b_out = np.asarray(b_out, dtype=np.float32)

    bf = ml_dtypes.bfloat16
    xT = np.ascontiguousarray(x.reshape(ROWS, DIM).T.astype(bf))
    wout_b = np.ascontiguousarray(w_out.astype(bf))
    bout2 = np.ascontiguousarray(b_out.reshape(1, DIM))

    in_maps = []
    for core in range(W):
        cols = [w_qkv[:, part * (HEADS * D) + core * HPC * D:
                      part * (HEADS * D) + (core + 1) * HPC * D]
                for part in range(3)]
        wq_c = np.ascontiguousarray(np.concatenate(cols, axis=1).astype(bf))
        in_maps.append({"xT": xT, "wq": wq_c, "wout": wout_b, "bout": bout2})

    trace = bool(os.environ.get("KERNEL_TRACE"))
    res = bass_utils.run_bass_kernel_spmd(
        nc, in_maps, core_ids=list(range(W)), trace=trace)
    last_exec_time_ns = res.exec_time_ns
    last_results = res

    out = np.concatenate([res.results[c]["out"] for c in range(W)], axis=0)
    return np.ascontiguousarray(out.reshape(B, N, DIM), dtype=np.float32)


# revision 30
# speedup vs baseline: 1.0138x; 1.0138x over previous
"""Trainium2 Bass kernel for nn_Attention_23476291240422 (sparse attention:
causal + 128-wide noncausal prefix block; b=4, n=2048, dim=2048, 16 heads,
d=128) distributed across 8 NeuronCores.

Sharding: head-parallel (2 heads/core) QKV projection + attention, AllToAll
(bf16) to flip head-sharded -> row-sharded, row-parallel out-proj (1024
rows/core). Inputs are staged to HBM as bf16 (the kernel computes in bf16
with fp32 PSUM accumulation). The AllToAll runs in two phases (seq-half
granularity) overlapped with attention/out-proj compute; stage 1 (projection)
is interleaved per batch with phase-A attention so ScalarE exp work hides
under TensorE projection matmuls.

kernel(x, w_qkv, w_out, b_out) -> [4, 2048, 2048] float32.
"""
import os
import sys
import types

import numpy as np
import ml_dtypes

import concourse.bass as bass
import concourse.mybir as mybir
import concourse.tile as tile
from concourse import bacc
from concourse import bass_utils

B, N, DIM = 4, 2048, 2048
HEADS, D, L = 16, 128, 128
W = 8
HPC = HEADS // W          # 2 heads per core
ROWS = B * N              # 8192
RPC = ROWS // W           # 1024 rows per core
SCALE = float(D) ** -0.5
P = 128
KT = DIM // P             # 16
S1CH = 512                # stage-1 seq chunk width
NBC = N // S1CH           # 4 stage-1 chunks per batch
CHW = 512                 # stage-2 i-chunk width / stage-4 col chunk
NJT = N // P              # 16 j-tiles
F32, BF16 = mybir.dt.float32, mybir.dt.bfloat16
PHASE_CHUNKS = ([1, 3], [0, 2])
M_BASE = (4, 0)  # stage-4 row-tile base per phase


def _install_ntff_hook():
    try:
        import antenv.axon_hooks  # noqa: F401
        return
    except ImportError:
        pass
    try:
        import antenv
        from trn_agent_boot.trn_boot import _ntff_profile_via_ctypes
        hook = [_ntff_profile_via_ctypes("/opt/axon/libaxon_pjrt.so")]
        mod = types.ModuleType("antenv.axon_hooks")
        mod.get_axon_ntff_profile_hook = lambda: hook[0]
        mod.set_axon_ntff_profile_hook = lambda h: hook.__setitem__(0, h)
        sys.modules["antenv.axon_hooks"] = mod
        antenv.axon_hooks = mod
    except Exception:
        pass


def build():
    nc = bacc.Bacc("TRN2", target_bir_lowering=False, debug=False, num_devices=W)

    xT = nc.dram_tensor("xT", [DIM, ROWS], BF16, kind="ExternalInput")
    wq = nc.dram_tensor("wq", [DIM, 6 * P], BF16, kind="ExternalInput")  # q0 q1 k0 k1 v0 v1
    wout = nc.dram_tensor("wout", [DIM, DIM], BF16, kind="ExternalInput")
    bout = nc.dram_tensor("bout", [1, DIM], F32, kind="ExternalInput")
    out = nc.dram_tensor("out", [RPC, DIM], F32, kind="ExternalOutput")

    tri_np = (np.arange(P)[:, None] <= np.arange(P)[None, :]).astype(ml_dtypes.bfloat16)
    tri = nc.inline_tensor(tri_np, name="tri")
    ident = nc.inline_tensor(np.eye(P, dtype=ml_dtypes.bfloat16), name="ident")

    def s1_batch(b, wq_bf, qk_b, v_dram_b, s1xf, s1ps, s1v_pool):
        xT_r = xT.rearrange("(kt p) n -> p kt n", p=P)
        for c in range(NBC):
            seq0 = b * N + c * S1CH
            xb = s1xf.tile([P, KT, S1CH], BF16, tag="xb")
            if b == 0 and c == 0:
                for kq in range(4):
                    nc.sync.dma_start(
                        xb[:, 4 * kq:4 * (kq + 1)],
                        xT_r[:, 4 * kq:4 * (kq + 1), seq0:seq0 + S1CH])
            else:
                nc.sync.dma_start(xb[:], xT_r[:, :, seq0:seq0 + S1CH])
            for m in range(4):
                ps = s1ps.tile([P, CHW], F32, tag="ps1")
                for kt in range(KT):
                    nc.tensor.matmul(
                        ps[:, :S1CH], wq_bf[:, kt, m * P:(m + 1) * P], xb[:, kt],
                        start=(kt == 0), stop=(kt == KT - 1))
                nc.vector.tensor_copy(
                    qk_b[:, m, c * S1CH:(c + 1) * S1CH], ps[:, :S1CH])
            for st2 in range(S1CH // P):
                st = c * (S1CH // P) + st2  # seq-tile within batch (0..15)
                psv = s1ps.tile([P, CHW], F32, tag="ps1")
                for kt in range(KT):
                    nc.tensor.matmul(
                        psv[:, :HPC * P], xb[:, kt, st2 * P:(st2 + 1) * P],
                        wq_bf[:, kt, 4 * P:6 * P],
                        start=(kt == 0), stop=(kt == KT - 1))
                vst = s1v_pool.tile([P, HPC * P], BF16, tag="vst")
                nc.vector.tensor_copy(vst[:], psv[:, :HPC * P])
                nc.sync.dma_start(v_dram_b[st * P:(st + 1) * P, :], vst[:])

    def s2_batch_phase(b, chunks, qk_b, v_dram_b, a2a_in_ph, pools,
                       tri_sb, ident_sb):
        s2va, s2pt, s2sm, s2ps, s2att, s2tp = pools
        v_r = v_dram_b.rearrange("(jt p) d -> p jt d", p=P)
        for hl in range(HPC):
            va = s2va.tile([P, NJT, P + 1], BF16, tag="va")
            nc.vector.memset(va[:, :, P:P + 1], 1.0)
            nc.sync.dma_start(va[:, :, :P], v_r[:, :, hl * P:(hl + 1) * P])
            qT = qk_b[:, hl]
            kTt = qk_b[:, 2 + hl]
            for c in chunks:
                pt = s2pt.tile([P, NJT, CHW], BF16, tag="pt")
                for J in range(4 * c + 4):
                    k_off = max(0, J - 4 * c)
                    nn_ = CHW - P * k_off
                    i0 = c * CHW + P * k_off
                    pss = s2ps.tile([P, CHW], F32, tag="pss")
                    nc.tensor.matmul(
                        pss[:, :nn_], kTt[:, J * P:(J + 1) * P],
                        qT[:, i0:(c + 1) * CHW], start=True, stop=True)
                    nc.scalar.activation(
                        pt[:, J, P * k_off:], pss[:, :nn_],
                        mybir.ActivationFunctionType.Exp, scale=SCALE)
                    if J >= 4 * c and not (c == 0 and J == 0):
                        nc.vector.tensor_mul(
                            pt[:, J, P * k_off:P * (k_off + 1)],
                            pt[:, J, P * k_off:P * (k_off + 1)], tri_sb[:])
                for pp in range(4):
                    it = 4 * c + pp
                    att = s2att.tile([P, P + 1], F32, tag="att")
                    for J in range(it + 1):
                        nc.tensor.matmul(
                            att[:], pt[:, J, P * pp:P * (pp + 1)], va[:, J],
                            start=(J == 0), stop=(J == it))
                    recip = s2sm.tile([P, 1], F32, tag="recip")
                    nc.vector.reciprocal(recip[:], att[:, P:P + 1])
                    attn = s2sm.tile([P, P], BF16, tag="attn")
                    nc.vector.tensor_scalar_mul(attn[:], att[:, :P], recip[:])
                    attT_ps = s2tp.tile([P, P], BF16, tag="attTps")
                    nc.tensor.transpose(attT_ps[:], attn[:], ident_sb[:])
                    attnT = s2sm.tile([P, P], BF16, tag="attnT")
                    nc.vector.tensor_copy(attnT[:], attT_ps[:])
                    dest = b * 2 + it // 8
                    cc = pp * P
                    last_w = nc.sync.dma_start(
                        a2a_in_ph[dest, hl * P:(hl + 1) * P, cc:cc + P],
                        attnT[:])
        return last_w

    with tile.TileContext(nc) as tc:
        with (
            tc.tile_pool(name="persist", bufs=1) as persist,
            tc.tile_pool(name="dram", bufs=1, space="DRAM") as dram,
        ):
            tri_sb = persist.tile([P, P], BF16)
            ident_sb = persist.tile([P, P], BF16)
            bout_sb = persist.tile([P, DIM], F32)

            v_drams = [dram.tile([N, HPC * P], BF16, name=f"v_dram{b}")
                       for b in range(B)]
            a2a_in = [dram.tile([W, HPC * P, RPC // 2], BF16, name=f"a2a_in{i}")
                      for i in range(2)]
            a2a_out = [dram.tile([W, HPC * P, RPC // 2], BF16, name=f"a2a_out{i}")
                       for i in range(2)]

            woutb_r = wout.rearrange("(kt p) c -> p kt c", p=P)

            with (
                tc.tile_pool(name="qk", bufs=1) as qkpool,
                tc.tile_pool(name="s2va", bufs=3) as s2va,
                tc.tile_pool(name="s2pt", bufs=2) as s2pt,
                tc.tile_pool(name="s2sm", bufs=8) as s2sm,
                tc.tile_pool(name="s2ps", bufs=2, space="PSUM") as s2ps,
                tc.tile_pool(name="s2att", bufs=2, space="PSUM") as s2att,
                tc.tile_pool(name="s2tp", bufs=2, space="PSUM") as s2tp,
            ):
                qk_bs = [qkpool.tile([P, 4, N], BF16, name=f"qkb{b}")
                         for b in range(B)]
                s2pools = (s2va, s2pt, s2sm, s2ps, s2att, s2tp)

                with (
                    tc.tile_pool(name="s1w", bufs=1) as s1w,
                    tc.tile_pool(name="s1v", bufs=3) as s1v_pool,
                    tc.tile_pool(name="s1xf", bufs=3) as s1xf,
                    tc.tile_pool(name="s1ps", bufs=2, space="PSUM") as s1ps,
                ):
                    wq_bf = s1w.tile([P, KT, 6 * P], BF16)
                    wq_r = wq.rearrange("(kt p) c -> p kt c", p=P)
                    nc.sync.dma_start(wq_bf[:, 0:4], wq_r[:, 0:4])
                    nc.sync.dma_start(tri_sb[:], tri.ap())
                    nc.sync.dma_start(ident_sb[:], ident.ap())
                    for kq in range(1, 4):
                        nc.sync.dma_start(
                            wq_bf[:, 4 * kq:4 * (kq + 1)],
                            wq_r[:, 4 * kq:4 * (kq + 1)])

                    # interleave: s1(b) then phase-A attention of batch b
                    for b in range(B):
                        s1_batch(b, wq_bf, qk_bs[b], v_drams[b],
                                 s1xf, s1ps, s1v_pool)
                        s2_batch_phase(b, PHASE_CHUNKS[0], qk_bs[b],
                                       v_drams[b], a2a_in[0], s2pools,
                                       tri_sb, ident_sb)

                nc.gpsimd.collective_compute(
                    "AllToAll", mybir.AluOpType.bypass,
                    replica_groups=[list(range(W))],
                    ins=[a2a_in[0][:].opt()], outs=[a2a_out[0][:].opt()],
                )

                with (
                    tc.tile_pool(name="s4l", bufs=2) as s4l,
                    tc.tile_pool(name="s4w", bufs=2) as s4w,
                    tc.tile_pool(name="s4o", bufs=4) as s4o,
                ):
                    # wb0/bias have no collective deps: load early.
                    wb0 = s4w.tile([P, KT, CHW], BF16, tag="wb")
                    nc.sync.dma_start(wb0[:], woutb_r[:, :, 0:CHW])
                    nc.sync.dma_start(
                        bout_sb[:], bout.ap().to_broadcast((P, DIM)))

                    from concourse.tile import add_dep_helper
                    last_writes = []
                    lhs0 = None
                    for b in range(B):
                        lw = s2_batch_phase(b, PHASE_CHUNKS[1], qk_bs[b],
                                            v_drams[b], a2a_in[1], s2pools,
                                            tri_sb, ident_sb)
                        last_writes.append(lw)
                        if b == B - 2:
                            # lhs0 (waits collective A) pinned after batch-2's
                            # DMAs so it cannot head-of-line-block phase B.
                            lhs0 = s4l.tile([P, KT, RPC // 2], BF16, tag="lhs")
                            l0 = nc.sync.dma_start(
                                lhs0[:],
                                a2a_out[0].rearrange("w h r -> (w h) r")
                                .rearrange("(kt p) r -> p kt r", p=P))
                            add_dep_helper(
                                l0.ins, lw.ins, sync=False,
                                reason="order lhs0 after phase-B b2")
                    wb1 = s4w.tile([P, KT, CHW], BF16, tag="wb")
                    nc.sync.dma_start(wb1[:], woutb_r[:, :, CHW:2 * CHW])
                    nc.gpsimd.collective_compute(
                        "AllToAll", mybir.AluOpType.bypass,
                        replica_groups=[list(range(W))],
                        ins=[a2a_in[1][:].opt()], outs=[a2a_out[1][:].opt()],
                    )


                    # ======== stage 4: out projection (2 phases) ========
                    with tc.tile_pool(name="s4ps", bufs=2,
                                      space="PSUM") as s4ps:
                        def s4_pass(phase, lhs, wbs):
                            last_out = None
                            for ncx in range(4):
                                if ncx < len(wbs):
                                    wb = wbs[ncx]
                                else:
                                    wb = s4w.tile([P, KT, CHW], BF16, tag="wb")
                                    nc.sync.dma_start(
                                        wb[:],
                                        woutb_r[:, :, ncx * CHW:(ncx + 1) * CHW])
                                for ml in range(4):
                                    m = M_BASE[phase] + ml
                                    ps4 = s4ps.tile([P, CHW], F32, tag="ps4")
                                    for kt in range(KT):
                                        nc.tensor.matmul(
                                            ps4[:],
                                            lhs[:, kt, ml * P:(ml + 1) * P],
                                            wb[:, kt],
                                            start=(kt == 0),
                                            stop=(kt == KT - 1))
                                    osb = s4o.tile([P, CHW], F32, tag="osb")
                                    nc.vector.tensor_tensor(
                                        osb[:], ps4[:],
                                        bout_sb[:, ncx * CHW:(ncx + 1) * CHW],
                                        mybir.AluOpType.add)
                                    last_out = nc.sync.dma_start(
                                        out[m * P:(m + 1) * P,
                                            ncx * CHW:(ncx + 1) * CHW],
                                        osb[:])
                            return last_out

                        last_out0 = s4_pass(0, lhs0, [wb0, wb1])
                        # preload phase-1's first weight chunk ahead of the
                        # collective-dependent lhs1 loads in the sync FIFO
                        wb10 = s4w.tile([P, KT, CHW], BF16, tag="wb")
                        nc.sync.dma_start(wb10[:], woutb_r[:, :, 0:CHW])
                        # lhs1 (waits collective B) pinned behind phase-0's
                        # final output write so it cannot block the sync FIFO
                        lhs1 = s4l.tile([P, KT, RPC // 2], BF16, tag="lhs")
                        a2a1_r = (a2a_out[1].rearrange("w h r -> (w h) r")
                                  .rearrange("(kt p) r -> p kt r", p=P))
                        for mlq in range(4):
                            l1 = nc.sync.dma_start(
                                lhs1[:, :, mlq * P:(mlq + 1) * P],
                                a2a1_r[:, :, mlq * P:(mlq + 1) * P])
                            add_dep_helper(
                                l1.ins, last_out0.ins, sync=False,
                                reason="order lhs1 after stage-4A outs")
                        s4_pass(1, lhs1, [wb10])

    nc.compile()
    return nc


_NC = None


def _get_nc():
    global _NC
    if _NC is None:
        _NC = build()
    return _NC


last_exec_time_ns = None
last_results = None


def kernel(x, w_qkv, w_out, b_out):
    global last_exec_time_ns, last_results
    _install_ntff_hook()
    nc = _get_nc()

    x = np.asarray(x, dtype=np.float32)
    w_qkv = np.asarray(w_qkv, dtype=np.float32)
    w_out = np.asarray(w_out, dtype=np.float32)
    b_out = np.asarray(b_out, dtype=np.float32)

    bf = ml_dtypes.bfloat16
    xT = np.ascontiguousarray(x.reshape(ROWS, DIM).T.astype(bf))
    wout_b = np.ascontiguousarray(w_out.astype(bf))
    bout2 = np.ascontiguousarray(b_out.reshape(1, DIM))

    in_maps = []
    for core in range(W):
        cols = [w_qkv[:, part * (HEADS * D) + core * HPC * D:
                      part * (HEADS * D) + (core + 1) * HPC * D]
                for part in range(3)]
        wq_c = np.ascontiguousarray(np.concatenate(cols, axis=1).astype(bf))
        in_maps.append({"xT": xT, "wq": wq_c, "wout": wout_b, "bout": bout2})

    trace = bool(os.environ.get("KERNEL_TRACE"))
    res = bass_utils.run_bass_kernel_spmd(
        nc, in_maps, core_ids=list(range(W)), trace=trace)
    last_exec_time_ns = res.exec_time_ns
    last_results = res

    out = np.concatenate([res.results[c]["out"] for c in range(W)], axis=0)
    return np.ascontiguousarray(out.reshape(B, N, DIM), dtype=np.float32)
